# revision 1
# baseline (speedup 1.0000x reference)
"""BERT input representation kernel for 8 TRN2 NeuronCores.

Math (reference):
    x1  = x @ W_emb + b_emb                      # [B,S,D]
    seg = einsum('bnsd,s->bnd', x1.reshape(B,S/8,8,D), w_seg) + b_seg
    out = (x1.reshape(...) + seg[:,:,None,:]).reshape(B,S,D) + PE(S,D)

Folded form used here (exact algebra):
    out[b,s,:] = (A @ x[b])[s,:] @ W_emb + bias[s,:]
where A = I + blockdiag(ones(8,1) @ w_seg[None,:]) mixes rows within each
8-row segment, and bias[s,:] = PE[s,:] + b_emb*(1 + sum(w_seg)) + b_seg.

Sharding: pure data-parallel over batch; each of 8 cores handles 8
batches (4096 rows = 32 row-tiles of 128 rows = 16 tile-pair groups).
The kernel is HBM-bandwidth-bound (the 16 MiB/core f32 output write
dominates), so the schedule is built to keep the store stream dense:

  - x is staged host-side into a DMA-friendly layout ([128, 32*64],
    partition p = row p of every tile) in bf16, the kernel's compute
    precision; W / bias / A^T are tiny host-prepared constants (bf16)
  - DMA issue order is arranged so the transfers that gate the first
    matmuls (A^T, first x wave, W) complete first, and the first output
    store launches ~4 us after the engines come up
  - per wave: one x-chunk load, then PE builds x~^T for a pair of
    row-tiles per matmul (psum[128,128] = [x_i|x_j].T @ A^T, which is
    the transpose + segment-mix in one op), ACT copies it to a resident
    bf16 tile
  - per tile-pair group: four K=64 matmuls (u=0 on PE rows 0-63, u=1 on
    rows 64-127 via a duplicated W copy, so pairs can execute
    concurrently on disjoint row groups), then DVE does the four
    bias-add + PSUM->SBUF drains; each row-tile ships as its own
    512 KiB store on the sync HWDGE ring (so a store gates on only two
    adds), with the final two stores on the scalar ring to drain the
    tail in parallel
"""

import sys

if "/opt/trn_rl_repo" not in sys.path:
    sys.path.insert(0, "/opt/trn_rl_repo")

import ml_dtypes
import numpy as np

import concourse.bacc as bacc
import concourse.mybir as mybir
import concourse.tile as tile
from concourse.bass_utils import run_bass_kernel_spmd

B, S, F, D, SEG = 64, 512, 64, 1024, 8
N_CORES = 8
B_LOC = B // N_CORES          # batches per core
ROWS = B_LOC * S              # 4096 rows per core
TILE_P = 128                  # rows per tile
N_TILES = ROWS // TILE_P      # 32
N_PAIR = N_TILES // 2         # 16 tile-pairs
N_BIAS = S // TILE_P          # 4 distinct bias row-tiles
HD = D // 2                   # 512

_NC_CACHE = None


def _build_nc():
    nc = bacc.Bacc("TRN2", target_bir_lowering=False, debug=False,
                   num_devices=N_CORES)
    # x pre-rearranged on host (layout + cast to the kernel's bf16
    # compute precision): xr[p, i*F:(i+1)*F] = x[i*128+p]
    # cols [0:128] = A^T, then the rearranged x — one first-wave DMA
    # covers both, with a single completion receipt on the head chain
    x_d = nc.declare_dram_parameter("x", [TILE_P, TILE_P + N_TILES * F],
                                    mybir.dt.bfloat16, isOutput=False)
    # combined constants [128, 5120]: cols [0:1024]=W stacked twice
    # (partitions 0-63 and 64-127 both hold W, so mains with lhsT at
    # base_partition 64 have a matching-base rhs) | [1024:3072]=bias0,bias1
    # | [3072:5120]=bias2,bias3
    cc_d = nc.declare_dram_parameter("cc", [TILE_P, 5 * D],
                                     mybir.dt.bfloat16, isOutput=False)
    out_d = nc.declare_dram_parameter("out", [ROWS, D], mybir.dt.float32,
                                      isOutput=True)

    with tile.TileContext(nc) as tc:
        with (
            tc.tile_pool(name="const", bufs=1) as cpool,
            tc.tile_pool(name="xbf", bufs=2) as xbpool,
            tc.tile_pool(name="outp", bufs=10) as opool,
            tc.tile_pool(name="ps_t", bufs=2, space="PSUM") as pst,
            tc.tile_pool(name="ps_o", bufs=6, space="PSUM") as pso,
        ):
            # A^T + the first x pair arrive in one persistent first-wave
            # load on the sync ring; the combined consts go on the scalar
            # ring in need-order (W first, bias pairs deferred into the
            # wave loop so their completion thresholds don't transitively
            # gate the first matmuls).
            at_x0 = cpool.tile([TILE_P, 2 * TILE_P], mybir.dt.bfloat16)
            nc.sync.dma_start(at_x0[:], x_d[:, 0:2 * TILE_P])
            at_ap = at_x0[:, 0:TILE_P]
            cc_sb = cpool.tile([TILE_P, 5 * D], mybir.dt.bfloat16)

            def bias_ap(jb, lo, hi):
                col = (jb + 1) * D
                return cc_sb[:, col + lo:col + hi]

            def w_ap(u, lo, hi):
                return cc_sb[64 * u:64 * u + F, lo:hi]

            # resident x~^T (bf16): xt_sb[64u+f, 128*pr+n] = x~[2pr+u, n, f]
            xt_sb = cpool.tile([TILE_P, N_PAIR * TILE_P], mybir.dt.bfloat16)

            # waves: load an x chunk, build x~^T for the wave's pairs
            # (phase 1), then matmul+bias+store those groups (phase 2).
            # Small first waves so the first output store launches early.
            WAVES = [1, 1, 2, 4, 4, 4]
            pr0 = 0
            for wn, wp in enumerate(WAVES):
                c0, cw = pr0 * TILE_P, wp * TILE_P   # x cols of this wave
                if wn == 0:
                    xcb = at_x0[:, TILE_P:2 * TILE_P]
                else:
                    xcb = xbpool.tile([TILE_P, 512], mybir.dt.bfloat16,
                                      name="xcb", tag="xcb")
                    nc.scalar.dma_start(xcb[:, 0:cw],
                                        x_d[:, TILE_P + c0:TILE_P + c0 + cw])
                if wn == 0:
                    nc.scalar.dma_start(cc_sb[:, 0:D], cc_d[:, 0:D])

                ps_x = pst.tile([TILE_P, 512], mybir.dt.float32,
                                name="ps_x", tag="ps_x")
                for k in range(wp):
                    nc.tensor.matmul(ps_x[:, 128 * k:128 * (k + 1)],
                                     xcb[:, 128 * k:128 * (k + 1)],
                                     at_ap, start=True, stop=True)
                nc.scalar.copy(xt_sb[:, c0:c0 + cw], ps_x[:, 0:cw])
                if wn == 0:
                    nc.scalar.dma_start(cc_sb[:, D:3 * D], cc_d[:, D:3 * D])
                elif wn == 1:
                    nc.scalar.dma_start(cc_sb[:, 3 * D:5 * D],
                                        cc_d[:, 3 * D:5 * D])

                for j in range(pr0, pr0 + wp):
                    # four single-bank PSUM quarters per group: [u][half]
                    q = [[pso.tile([TILE_P, HD], mybir.dt.float32,
                                   name=f"q{u}{h}", tag="q")
                          for h in range(2)] for u in range(2)]
                    jbs = ((2 * j) % N_BIAS, (2 * j + 1) % N_BIAS)
                    lhss = tuple(
                        xt_sb[64 * u:64 * (u + 1), 128 * j:128 * (j + 1)]
                        for u in range(2))
                    # mains; u=0 uses PE rows 0-63, u=1 rows 64-127 (these
                    # can execute concurrently on disjoint row groups)
                    for u in range(2):
                        nc.tensor.matmul(q[u][0][:], lhss[u],
                                         w_ap(u, 0, HD),
                                         start=True, stop=True)
                    for u in range(2):
                        nc.tensor.matmul(q[u][1][:], lhss[u],
                                         w_ap(u, HD, D),
                                         start=True, stop=True)
                    # bias add + PSUM drain, all on DVE (ACT/GpSimd
                    # variants measured slower — DVE port contention);
                    # one 512 KiB store per row-tile so each store gates
                    # on just two adds
                    for u in range(2):
                        i = 2 * j + u
                        o_t = opool.tile([TILE_P, D], mybir.dt.float32,
                                         name="o_t")
                        nc.vector.tensor_add(o_t[:, 0:HD], q[u][0][:],
                                             bias_ap(jbs[u], 0, HD))
                        nc.vector.tensor_add(o_t[:, HD:D], q[u][1][:],
                                             bias_ap(jbs[u], HD, D))
                        rows = out_d[i * TILE_P:(i + 1) * TILE_P, :]
                        if i == 0:
                            nc.sync.dma_start(rows[:, 0:HD], o_t[:, 0:HD])
                            nc.sync.dma_start(rows[:, HD:D], o_t[:, HD:D])
                        else:
                            eng = (nc.scalar if i >= N_TILES - 2
                                   else nc.sync)
                            eng.dma_start(rows, o_t[:])
                pr0 += wp
    nc.compile()
    return nc


def _host_constants(W_emb, b_emb, w_seg, b_seg):
    # sinusoidal positional encoding, float32, same formula as the reference
    pos = np.arange(S, dtype=np.float32)[:, None]
    div = np.exp(np.arange(0, D, 2, dtype=np.float32)
                 * (-np.log(10000.0) / D)).astype(np.float32)
    ang = pos * div
    pe = np.zeros((S, D), np.float32)
    pe[:, 0::2] = np.sin(ang)
    pe[:, 1::2] = np.cos(ang)

    bias = (pe + b_emb[None, :] * (np.float32(1.0) + w_seg.sum())
            + b_seg[0]).astype(np.float32)
    # rearrange to [128, 4*D]: column block j holds bias rows j*128..j*128+127
    bias_r = np.ascontiguousarray(
        bias.reshape(N_BIAS, TILE_P, D).transpose(1, 0, 2).reshape(
            TILE_P, N_BIAS * D)).astype(ml_dtypes.bfloat16)

    blk = np.eye(SEG, dtype=np.float32) + w_seg[:, None] * np.ones(
        (1, SEG), np.float32)
    at = np.kron(np.eye(TILE_P // SEG, dtype=np.float32), blk).astype(
        ml_dtypes.bfloat16)

    wb = np.vstack([W_emb, W_emb]).astype(ml_dtypes.bfloat16)
    # combined consts: [bias0|bias1|W2|bias2|bias3] as [128, 5*D] bf16
    cc = np.ascontiguousarray(np.concatenate([wb, bias_r], axis=1))
    return at, cc


def _prepare_in_maps(x, W_emb, b_emb, w_seg, b_seg):
    x = np.ascontiguousarray(np.asarray(x, dtype=np.float32))
    W_emb = np.asarray(W_emb, dtype=np.float32)
    b_emb = np.asarray(b_emb, dtype=np.float32)
    w_seg = np.asarray(w_seg, dtype=np.float32)
    b_seg = np.asarray(b_seg, dtype=np.float32)

    at, cc = _host_constants(W_emb, b_emb, w_seg, b_seg)

    in_maps = []
    for c in range(N_CORES):
        xs = x[c * B_LOC:(c + 1) * B_LOC].reshape(ROWS, F)
        # rearrange [32 tiles, 128 rows, F] -> [128, 32*F], bf16 staging
        xr = np.ascontiguousarray(
            xs.reshape(N_TILES, TILE_P, F).transpose(1, 0, 2).reshape(
                TILE_P, N_TILES * F)).astype(ml_dtypes.bfloat16)
        in_maps.append(
            {"x": np.ascontiguousarray(np.concatenate([at, xr], axis=1)),
             "cc": cc})
    return in_maps


def kernel(x, W_emb, b_emb, w_seg, b_seg):
    in_maps = _prepare_in_maps(x, W_emb, b_emb, w_seg, b_seg)

    global _NC_CACHE
    if _NC_CACHE is None:
        _NC_CACHE = _build_nc()

    res = run_bass_kernel_spmd(_NC_CACHE, in_maps,
                               core_ids=list(range(N_CORES)))
    out = np.concatenate(
        [np.asarray(res.results[c]["out"]).reshape(B_LOC, S, D)
         for c in range(N_CORES)], axis=0)
    return out



# revision 2
# speedup vs baseline: 1.2736x; 1.2736x over previous
"""BERT input representation kernel for 8 TRN2 NeuronCores.

Math (reference):
    x1  = x @ W_emb + b_emb                      # [B,S,D]
    seg = einsum('bnsd,s->bnd', x1.reshape(B,S/8,8,D), w_seg) + b_seg
    out = (x1.reshape(...) + seg[:,:,None,:]).reshape(B,S,D) + PE(S,D)

Folded form used here (exact algebra):
    out[b,s,:] = (M @ x[b])[s,:] @ W_emb + bias[s,:]
where M = I + blockdiag(ones(8,1) @ w_seg[None,:]) mixes rows within each
8-row segment, and bias[s,:] = PE[s,:] + b_emb*(1 + sum(w_seg)) + b_seg.

Key optimization: the bias matrix, viewed per 128-row tile (4 distinct
tiles, s-period 512), factors as bias_tile[tb] = g @ V_tb with a SHARED
within-tile basis g [128, 64] — numerical rank of the [128, 4*1024]
reshape is ~48 (sinusoidal PE splits into tile-phase x within-tile
sinusoids). So the bias rides the main matmul as 64 extra K-rows
(TensorE cost is N-dependent, not K-dependent), the PSUM drain becomes a
pure copy (no tensor_tensor add), and the copies split across DVE and
ACT.  Output is stored bf16 (tolerance 2e-2 >> bf16 rounding 1.7e-3) and
upcast to f32 on the host, halving the dominant HBM store traffic.

Per core (8 batches = 4096 rows = 32 tiles of 128):
  - transpose-mix: per tile, matmul(lhsT=x chunk [128,64], rhs=M^T
    [128,128]) -> PSUM [64,128] = (M x)^T; ACT drains 4-tile chunks into
    lhs_sb partitions 0-63 (bf16)
  - lhs_sb bottom partitions 64-127 hold g^T, replicated to all 32 tile
    column blocks by DVE doubling copies (one 16 KiB DMA seed)
  - mains: per tile, 2 matmuls K=128 N=512: lhsT = lhs_sb block
    ([xt ; g^T]), rhs = wv block ([W ; V_tb]); W is replicated on-chip
    (DVE) to pair with the 4 V blocks, V loads once
  - drain: even tiles on DVE, odd on ACT, pure f32->bf16 copies into a
    [128, 2048] pair buffer; one 512 KiB store per pair (4 KiB per
    partition -> full-size DMA descriptors), alternating sync/scalar
    rings
"""

import sys

if "/opt/trn_rl_repo" not in sys.path:
    sys.path.insert(0, "/opt/trn_rl_repo")

import ml_dtypes
import numpy as np

import concourse.bacc as bacc
import concourse.mybir as mybir
import concourse.tile as tile
from concourse.bass_utils import run_bass_kernel_spmd

B, S, F, D, SEG = 64, 512, 64, 1024, 8
N_CORES = 8
B_LOC = B // N_CORES          # batches per core
ROWS = B_LOC * S              # 4096 rows per core
TILE_P = 128                  # rows per tile
N_TILES = ROWS // TILE_P      # 32
N_PAIR = N_TILES // 2         # 16
N_BIAS = S // TILE_P          # 4 distinct bias row-tiles
RNK = 64                      # bias factorization rank
HD = D // 2                   # 512

W_TILES = [2, 2, 4, 8, 8, 8]  # tiles per wave

_NC_CACHE = None


def _build_nc():
    nc = bacc.Bacc("TRN2", target_bir_lowering=False, debug=False,
                   num_devices=N_CORES)
    # xc: cols [0:128] = M^T, then x rearranged [128, 32*64]
    # (xr[p, t*F:(t+1)*F] = x[t*128+p]), all bf16
    xc_d = nc.declare_dram_parameter("xc", [TILE_P, TILE_P + N_TILES * F],
                                     mybir.dt.bfloat16, isOutput=False)
    # cc: [64, 128+1024+4096] = [g^T | W | V]
    cc_d = nc.declare_dram_parameter("cc", [RNK, TILE_P + D + N_BIAS * D],
                                     mybir.dt.bfloat16, isOutput=False)
    # out: pair layout [128, 16*2048]; block p = [tile 2p | tile 2p+1],
    # partition q = row q within the tile; host unscrambles + upcasts
    out_d = nc.declare_dram_parameter("out", [TILE_P, N_TILES * D],
                                      mybir.dt.bfloat16, isOutput=True)

    with tile.TileContext(nc) as tc:
        with (
            tc.tile_pool(name="const", bufs=1) as cpool,
            tc.tile_pool(name="xb", bufs=2) as xbpool,
            tc.tile_pool(name="outp", bufs=6) as opool,
            tc.tile_pool(name="ps_t", bufs=2, space="PSUM") as pst,
            tc.tile_pool(name="ps_o", bufs=3, space="PSUM") as pso,
        ):
            # head: M^T + the first wave's x, one sync-ring DMA
            head = cpool.tile([TILE_P, TILE_P + W_TILES[0] * F],
                              mybir.dt.bfloat16)
            nc.sync.dma_start(head[:], xc_d[:, 0:TILE_P + W_TILES[0] * F])
            at_ap = head[:, 0:TILE_P]

            # lhs_sb block t: partitions 0-63 = (M x)^T features of tile
            # t, partitions 64-127 = g^T (replicated)
            lhs_sb = cpool.tile([TILE_P, N_TILES * TILE_P],
                                mybir.dt.bfloat16)
            # wv block tb: partitions 0-63 = W, 64-127 = V_tb
            wv_sb = cpool.tile([TILE_P, N_BIAS * D], mybir.dt.bfloat16)

            # const loads: g seed (sync, tiny), W + V halves (scalar)
            nc.sync.dma_start(lhs_sb[64:128, 0:TILE_P], cc_d[:, 0:TILE_P])
            nc.scalar.dma_start(wv_sb[0:64, 0:D],
                                cc_d[:, TILE_P:TILE_P + D])
            nc.scalar.dma_start(wv_sb[64:128, 0:2 * D],
                                cc_d[:, TILE_P + D:TILE_P + 3 * D])
            nc.scalar.dma_start(wv_sb[64:128, 2 * D:4 * D],
                                cc_d[:, TILE_P + 3 * D:TILE_P + 5 * D])

            # on-chip replication (DVE): g^T across all 32 blocks
            # (doubling), W across the 4 wv blocks
            span = TILE_P
            while span < N_TILES * TILE_P:
                w = min(span, N_TILES * TILE_P - span)
                nc.vector.tensor_copy(lhs_sb[64:128, span:span + w],
                                      lhs_sb[64:128, 0:w])
                span += w
            nc.vector.tensor_copy(wv_sb[0:64, D:2 * D], wv_sb[0:64, 0:D])
            nc.vector.tensor_copy(wv_sb[0:64, 2 * D:4 * D],
                                  wv_sb[0:64, 0:2 * D])

            o_pair = None
            pr = 0
            for wn, wp in enumerate(W_TILES):
                if wn == 0:
                    xcb = head[:, TILE_P:TILE_P + wp * F]
                else:
                    xcb = xbpool.tile([TILE_P, 512], mybir.dt.bfloat16,
                                      name="xcb", tag="xcb")
                    nc.sync.dma_start(
                        xcb[:, 0:wp * F],
                        xc_d[:, TILE_P + pr * F:TILE_P + (pr + wp) * F])

                # transpose-mix in chunks of <=4 tiles, ACT drains
                for c0 in range(0, wp, 4):
                    k = min(4, wp - c0)
                    ps_x = pst.tile([64, 512], mybir.dt.float32,
                                    name="ps_x", tag="ps_x")
                    for i in range(k):
                        nc.tensor.matmul(ps_x[0:64, 128 * i:128 * (i + 1)],
                                         xcb[:, (c0 + i) * F:(c0 + i + 1) * F],
                                         at_ap, start=True, stop=True)
                    t0 = pr + c0
                    nc.scalar.copy(lhs_sb[0:64, 128 * t0:128 * (t0 + k)],
                                   ps_x[0:64, 0:128 * k])

                for i in range(wp):
                    t = pr + i
                    tb = t % N_BIAS
                    ps = pso.tile([TILE_P, D], mybir.dt.float32,
                                  name="ps", tag="ps")
                    lhsT = lhs_sb[:, 128 * t:128 * (t + 1)]
                    nc.tensor.matmul(ps[:, 0:HD], lhsT,
                                     wv_sb[:, tb * D:tb * D + HD],
                                     start=True, stop=True)
                    nc.tensor.matmul(ps[:, HD:D], lhsT,
                                     wv_sb[:, tb * D + HD:(tb + 1) * D],
                                     start=True, stop=True)
                    if t % 2 == 0:
                        o_pair = opool.tile([TILE_P, 2 * D],
                                            mybir.dt.bfloat16,
                                            name="o_pair", tag="op")
                        nc.vector.tensor_copy(o_pair[:, 0:D], ps[:])
                    else:
                        nc.scalar.copy(o_pair[:, D:2 * D], ps[:])
                        p = t // 2
                        eng = nc.sync if p % 2 == 0 else nc.scalar
                        eng.dma_start(out_d[:, 2048 * p:2048 * (p + 1)],
                                      o_pair[:])
                pr += wp
    nc.compile()
    return nc


def _host_constants(W_emb, b_emb, w_seg, b_seg):
    # sinusoidal positional encoding, float32, same formula as reference
    pos = np.arange(S, dtype=np.float32)[:, None]
    div = np.exp(np.arange(0, D, 2, dtype=np.float32)
                 * (-np.log(10000.0) / D)).astype(np.float32)
    ang = pos * div
    pe = np.zeros((S, D), np.float32)
    pe[:, 0::2] = np.sin(ang)
    pe[:, 1::2] = np.cos(ang)

    bias = (pe + b_emb[None, :] * (np.float32(1.0) + w_seg.sum())
            + b_seg[0]).astype(np.float64)
    # within-tile factorization: bias.reshape(4,128,D) -> [128, 4*D],
    # rank-64 SVD; V re-solved against the bf16-quantized g
    B_all = bias.reshape(N_BIAS, TILE_P, D).transpose(1, 0, 2).reshape(
        TILE_P, N_BIAS * D)
    U, sv, Vt = np.linalg.svd(B_all, full_matrices=False)
    g = (U[:, :RNK] * np.sqrt(sv[:RNK])).astype(
        ml_dtypes.bfloat16).astype(np.float64)
    V, *_ = np.linalg.lstsq(g, B_all, rcond=None)
    gT = np.ascontiguousarray(g.T).astype(ml_dtypes.bfloat16)  # [64, 128]
    Vb = np.ascontiguousarray(V).astype(ml_dtypes.bfloat16)    # [64, 4096]

    # M^T[p, n] = delta + w_seg[p % 8] within each 8-row segment
    blk = np.eye(SEG, dtype=np.float32) + w_seg[:, None] * np.ones(
        (1, SEG), np.float32)
    at = np.kron(np.eye(TILE_P // SEG, dtype=np.float32), blk).astype(
        ml_dtypes.bfloat16)

    Wb = W_emb.astype(ml_dtypes.bfloat16)                      # [64, 1024]
    cc = np.ascontiguousarray(np.concatenate([gT, Wb, Vb], axis=1))
    return at, cc


def _prepare_in_maps(x, W_emb, b_emb, w_seg, b_seg):
    x = np.ascontiguousarray(np.asarray(x, dtype=np.float32))
    W_emb = np.asarray(W_emb, dtype=np.float32)
    b_emb = np.asarray(b_emb, dtype=np.float32)
    w_seg = np.asarray(w_seg, dtype=np.float32)
    b_seg = np.asarray(b_seg, dtype=np.float32)

    at, cc = _host_constants(W_emb, b_emb, w_seg, b_seg)

    in_maps = []
    for c in range(N_CORES):
        xs = x[c * B_LOC:(c + 1) * B_LOC].reshape(ROWS, F)
        xr = np.ascontiguousarray(
            xs.reshape(N_TILES, TILE_P, F).transpose(1, 0, 2).reshape(
                TILE_P, N_TILES * F)).astype(ml_dtypes.bfloat16)
        in_maps.append(
            {"xc": np.ascontiguousarray(np.concatenate([at, xr], axis=1)),
             "cc": cc})
    return in_maps


def kernel(x, W_emb, b_emb, w_seg, b_seg):
    in_maps = _prepare_in_maps(x, W_emb, b_emb, w_seg, b_seg)

    global _NC_CACHE
    if _NC_CACHE is None:
        _NC_CACHE = _build_nc()

    res = run_bass_kernel_spmd(_NC_CACHE, in_maps,
                               core_ids=list(range(N_CORES)))
    out = np.concatenate(
        [np.asarray(res.results[c]["out"])
         .reshape(TILE_P, N_PAIR, 2, D).transpose(1, 2, 0, 3)
         .reshape(B_LOC, S, D).astype(np.float32)
         for c in range(N_CORES)], axis=0)
    return out
